# revision 34
# baseline (speedup 1.0000x reference)
"""Trainium2 Bass kernel for nn_Net_17532056502451.

5 "think" iterations: shift-window cosine selector (159 shifts) + softmax
attention + scatter-back + conv-style encoder/decoder with energy argmax
(81 shifts), masked-MSE losses averaged.  Data-parallel: 1024 tokens over
8 cores, 128 tokens/core (one per SBUF partition), token-major.
HW exec ~188 us vs ~1.44 ms for the v1 baseline (7.7x).

Design notes:
- dot correlation: fp16 broadcast-product in 3 zero-structure bands
  (xpad is nonzero only in [79,159)) at DVE 2x mode, then 2x fold-adds
  (80->40->20->10, edge bands are pure copies) + one small strided
  reduce.  TENSOR_REDUCE never runs 2x, TENSOR_TENSOR does.
- energy Gram form E = z @ Az + [ya;1] @ Atail with features packed
  6400 -> 4096 = 32 chunks of 128 (d<32: i<80, d<48: i<48, d<64: i<32,
  d<80: i<16); whole PE path fp16 (1-pass matmuls, FWL); transposes via
  regular matmul against a fp16 identity (faster than transpose-mode);
  E-matmul operands swapped (stationary = zT chunk, moving = Az chunk)
  so E accumulates TOKEN-major in PSUM and argmax reads PSUM directly.
- E-pipeline: chunk PAIRS share one [128,256] PSUM tile; PSUM->SBUF
  copies are 256 wide and split Scalar/Vector 12/4.
- per-token dynamic window gathers via gpsimd local_scatter with
  per-partition indices (idx[p,j] = j - start_p, negatives ignored);
  gathers read only the 80 nonzero source columns.  A dummy scatter at
  each iteration start absorbs the gpsimd library load (MODIFY_POOL_
  CONFIG + ~2us IRAM DMA) off the critical path.
- softmax without max-subtraction (exp args bounded ~21 here) and with
  ACT-fused accum_out for the denominator; approx reciprocals; loss via
  ACT Square with accum_out; argmax indices consumed directly from the
  uint32 max_index output as DVE scalars.
- all constants pre-swizzled on host into one fp16 blob + one fp32 blob
  (4 input DMAs total, split across Sync/ACT/gpsimd queues).
"""
import numpy as np

IDIM = 80
ODIM = 80
HDIM = 512
THINK_ITER = 5
TEMPER = 0.7
B, T = 4, 256
NTOK = B * T
P = 128
NCORES = 8
S1 = 159
S2 = 81
ZBLOCKS = [(0, 16, 80), (16, 32, 80), (32, 48, 48), (48, 64, 32), (64, 80, 16)]
NFEAT = sum((d1 - d0) * im for d0, d1, im in ZBLOCKS)   # 4096
NCHUNK = NFEAT // 128   # 32 z chunks
NCH = NCHUNK + 1        # +1 chunk holding [ya(80); 1; 0...]
NFE = NCH * 128         # 4224
# fp16 const blob column offsets
OF_A = 0
OF_M0 = OF_A + NCH * 81             # 2673
OF_M1 = OF_M0 + 160                 # 2833 (rows 0:32)
OF_ID = OF_M1 + 160                 # 2993
OF_IX = OF_ID + 128                 # 3121
W16 = OF_IX + 2                     # 3123
# fp32 const blob: bfused(2) iota(256)
OF_BS = 0
OF_IO = 2
W32 = 258

_cache = {}


def _feat_list():
    feats = []
    for d0, d1, im in ZBLOCKS:
        for d in range(d0, d1):
            for i in range(im):
                feats.append((d, i))
    return feats


def _build_consts(W_enc, b_enc, W_src, b_src):
    W_enc = np.asarray(W_enc, np.float32)
    b_enc = np.asarray(b_enc, np.float32)
    W_src = np.asarray(W_src, np.float32)
    b_src = np.asarray(b_src, np.float32)
    C = (W_enc.T @ W_enc).astype(np.float32)
    q = (W_enc.T @ b_enc).astype(np.float32)
    bb = np.float32(b_enc @ b_enc)
    feats = _feat_list()
    Az = np.zeros((S2, NFE), np.float32)
    for s in range(S2):
        dd = 80 - s
        for f, (d, i) in enumerate(feats):
            if i < 80 - d:
                Az[s, f] = (2.0 if d > 0 else 1.0) * C[dd + i, dd + i + d]
        # linear tail features [ya(80); 1] in chunk 32
        Az[s, NFEAT:NFEAT + 80] = 2.0 * q[dd:dd + 80]
        Az[s, NFEAT + 80] = bb
    c16 = np.zeros((P, W16), np.float16)
    # A: chunk k at cols OF_A + k*81, partition p holds Az.T[k*128+p, :]
    AzT = np.ascontiguousarray(Az.T).astype(np.float16)          # (4224, 81)
    c16[:, OF_A:OF_A + NCH * 81] = AzT.reshape(NCH, 128, 81) \
        .transpose(1, 0, 2).reshape(128, NCH * 81)
    # fused enc->dec map: x_ext = M @ yhat + bf
    M = (W_src @ W_enc).astype(np.float32)                       # (160, 160)
    bf = (W_src @ b_enc + b_src).astype(np.float32)              # (160,)
    MT = np.ascontiguousarray(M.T).astype(np.float16)            # (j, o)
    c16[:, OF_M0:OF_M0 + 160] = MT[0:128]
    c16[0:32, OF_M1:OF_M1 + 160] = MT[128:160]
    c16[:, OF_ID:OF_ID + 128] = np.eye(128, dtype=np.float16)
    c16[:, OF_IX:OF_IX + 2] = np.broadcast_to(
        np.array([0, 1], np.int16).view(np.float16), (P, 2))
    c32 = np.zeros((P, W32), np.float32)
    c32[:, OF_BS] = bf[0:128]
    c32[0:32, OF_BS + 1] = bf[128:160]
    c32[:, OF_IO:OF_IO + 256] = np.arange(256, dtype=np.float32)
    return dict(c16=c16, c32=c32)


def _make_in_maps(x, y, consts):
    xt = x.reshape(NTOK, IDIM)
    yt = y.reshape(NTOK, ODIM)
    in_maps = []
    for c in range(NCORES):
        m = dict(consts)
        m["xy"] = np.ascontiguousarray(
            np.concatenate([xt[c * P:(c + 1) * P], yt[c * P:(c + 1) * P]],
                           axis=1))
        in_maps.append(m)
    return in_maps


def _build_nc():
    import concourse.bass as bass
    import concourse.bacc as bacc
    import concourse.mybir as mybir
    from concourse.tile import TileContext

    F32 = mybir.dt.float32
    F16 = mybir.dt.float16
    I16 = mybir.dt.int16
    U32 = mybir.dt.uint32
    Op = mybir.AluOpType
    AF = mybir.ActivationFunctionType

    nc = bacc.Bacc()
    d_xy = nc.declare_dram_parameter("xy", [P, 160], F32, isOutput=False)
    d_c16 = nc.declare_dram_parameter("c16", [P, W16], F16, isOutput=False)
    d_c32 = nc.declare_dram_parameter("c32", [P, W32], F32, isOutput=False)
    d_out = nc.declare_dram_parameter("losspart", [P, 8], F32, isOutput=True)

    with TileContext(nc) as tc:
        with (
            tc.tile_pool(name="const", bufs=1) as cpool,
            tc.tile_pool(name="work", bufs=1) as pool,
            tc.tile_pool(name="zrot", bufs=8) as zpool,
            tc.tile_pool(name="ps_rot", bufs=4, space="PSUM") as pp,
            tc.tile_pool(name="ps_h", bufs=3, space="PSUM") as pph,
            tc.tile_pool(name="ps_acc", bufs=1, space="PSUM") as ppe,
        ):
            # ---- inputs + constants (3 DMAs) ----
            xy_t = pool.tile([P, 160], F32, tag="xy")
            nc.sync.dma_start(xy_t[:], d_xy[:])
            c16 = cpool.tile([P, W16], F16, tag="c16")
            HH = W16 // 2
            nc.gpsimd.dma_start(c16[:, 0:HH], d_c16[:, 0:HH])
            nc.scalar.dma_start(c16[:, HH:W16], d_c16[:, HH:W16])
            c32 = cpool.tile([P, W32], F32, tag="c32")
            nc.sync.dma_start(c32[:], d_c32[:])

            def Achunk(k):
                return c16[:, OF_A + k * 81:OF_A + (k + 1) * 81]
            id_t = c16[:, OF_ID:OF_ID + 128]
            bs_t = c32[:, OF_BS:OF_BS + 2]
            io_t = c32[:, OF_IO:OF_IO + 256]

            # ---- state ----
            xpad = pool.tile([P, 238], F32, tag="xpad")
            xpad16 = pool.tile([P, 238], F16, tag="xpad16")
            yres = pool.tile([P, 80], F32, tag="yres")
            keep = pool.tile([P, 80], F32, tag="keep")
            yap16 = pool.tile([P, 240], F16, tag="yap16")
            lossp = pool.tile([P, 8], F32, tag="lossp")
            nc.vector.memset(xpad[:], 0.0)
            nc.vector.memset(yap16[:], 0.0)
            nc.vector.memset(lossp[:], 0.0)
            nc.scalar.copy(xpad[:, 79:159], xy_t[:, 0:80])
            nc.vector.tensor_copy(yres[:], xy_t[:, 80:160])
            nc.vector.tensor_scalar(keep[:], yres[:], 0.0, None, Op.not_equal)

            sqx = pool.tile([P, 239], F32, tag="sqx")
            nc.vector.memset(sqx[:, 0:1], 0.0)
            cs = pool.tile([P, 239], F32, tag="cs")
            nsq = pool.tile([P, S1], F32, tag="nsq")
            rnsq = pool.tile([P, S1], F32, tag="rnsq")
            yres16 = pool.tile([P, 80], F16, tag="yres16")
            w2 = pool.tile([P, S1 * 80], F16, tag="w2")
            w4 = pool.tile([P, S1 * 40], F16, tag="w4")
            w5 = pool.tile([P, S1 * 20], F16, tag="w5")
            w6 = pool.tile([P, S1 * 10], F16, tag="w6")
            dot16 = pool.tile([P, S1], F16, tag="dot16")
            adot = pool.tile([P, S1], F16, tag="adot")
            gsel = pool.tile([P, S1], F32, tag="gsel")
            mx8 = pool.tile([P, 8], F32, tag="mx8")
            mi8 = pool.tile([P, 8], U32, tag="mi8")
            thf = pool.tile([P, 1], F32, tag="thf")
            th2 = pool.tile([P, 1], F32, tag="th2")
            sf = pool.tile([P, 1], F32, tag="sf")
            df = pool.tile([P, 1], F32, tag="df")
            ix1 = pool.tile([P, 80], I16, tag="ix1")
            ix2 = pool.tile([P, 80], I16, tag="ix2")
            ix3 = pool.tile([P, 80], I16, tag="ix3")
            ix4 = pool.tile([P, 160], I16, tag="ix4")
            yal = pool.tile([P, 256], F16, tag="yal")
            xele = pool.tile([P, 256], F16, tag="xele")
            yhat = pool.tile([P, 256], F16, tag="yhat")
            yele = pool.tile([P, 160], F16, tag="yele")
            zt = pool.tile([P, 80], F32, tag="zt")
            et = pool.tile([P, 80], F32, tag="et")
            ssum = pool.tile([P, 1], F32, tag="ssum")
            rsum = pool.tile([P, 1], F32, tag="rsum")
            nzm = pool.tile([P, 1], F32, tag="nzm")
            zero1 = pool.tile([P, 1], F32, tag="zero1")
            nc.vector.memset(zero1[:], 0.0)
            zf16 = pool.tile([P, NFE], F16, tag="zf16")
            nc.vector.memset(zf16[:, NFEAT:NFE], 0.0)
            nc.vector.memset(zf16[:, NFEAT + 80:NFEAT + 81], 1.0)
            yhT0 = pool.tile([128, 128], F16, tag="yhT0")
            yhT1 = pool.tile([32, 128], F16, tag="yhT1")
            xeT0 = pool.tile([128, 128], F16, tag="xeT0")
            xeT1 = pool.tile([32, 128], F16, tag="xeT1")
            xext16 = pool.tile([P, 160], F16, tag="xext16")
            dtmp = pool.tile([P, 80], F32, tag="dtmp")
            dsq = pool.tile([P, 80], F32, tag="dsq")
            gdum = pool.tile([P, 2], F16, tag="gdum")
            ixdum = c16[:, OF_IX:OF_IX + 2].bitcast(I16)

            def vap(tile_ap, free0, fdims):
                b = tile_ap
                return bass.AP(b.tensor, b.offset + free0,
                               [list(b.ap[0])] + list(fdims))

            def norms():
                nc.scalar.activation(sqx[:, 1:239], xpad[:], AF.Square)
                nc.vector.tensor_tensor_scan(cs[:], sqx[:],
                                             zero1[:].to_broadcast((P, 239)),
                                             0.0, Op.add, Op.bypass)
                nc.vector.tensor_tensor(nsq[:], cs[:, 80:239], cs[:, 0:159],
                                        Op.subtract)
                nc.vector.tensor_scalar_max(rnsq[:], nsq[:], 1e-30)
                nc.vector.reciprocal_approx_fast(rnsq[:], rnsq[:])

            norms()


            for it in range(THINK_ITER):
                # gpsimd library warm-up: absorbs the MODIFY_POOL_CONFIG +
                # IRAM load off the critical path while Vector runs the dot
                nc.gpsimd.local_scatter(gdum[:], c16[:, OF_IX:OF_IX + 2],
                                        ixdum, channels=128, num_elems=2,
                                        num_idxs=2)
                # --- dot: fp16 product (2x) + fold-adds + small reduce ---
                nc.scalar.copy(xpad16[:], xpad[:])
                nc.vector.tensor_copy(yres16[:], yres[:])
                # band E1: s in [0,40), c in [40,80)
                nc.vector.tensor_tensor(
                    vap(w2[:], 40, [[80, 40], [1, 40]]),
                    vap(xpad16[:], 40, [[1, 40], [1, 40]]),
                    vap(yres16[:], 40, [[0, 40], [1, 40]]), Op.mult)
                # band C: s in [40,119), full c
                nc.vector.tensor_tensor(
                    vap(w2[:], 40 * 80, [[80, 79], [1, 80]]),
                    vap(xpad16[:], 40, [[1, 79], [1, 80]]),
                    vap(yres16[:], 0, [[0, 79], [1, 80]]), Op.mult)
                # band E2: s in [119,159), c in [0,40)
                nc.vector.tensor_tensor(
                    vap(w2[:], 119 * 80, [[80, 40], [1, 40]]),
                    vap(xpad16[:], 119, [[1, 40], [1, 40]]),
                    vap(yres16[:], 0, [[0, 40], [1, 40]]), Op.mult)
                with nc.allow_low_precision("argmax-only dot"):
                    nc.vector.tensor_copy(
                        vap(w4[:], 0, [[40, 40], [1, 40]]),
                        vap(w2[:], 40, [[80, 40], [1, 40]]))
                    nc.vector.tensor_tensor(
                        vap(w4[:], 40 * 40, [[40, 79], [1, 40]]),
                        vap(w2[:], 40 * 80, [[80, 79], [1, 40]]),
                        vap(w2[:], 40 * 80 + 40, [[80, 79], [1, 40]]), Op.add)
                    nc.vector.tensor_copy(
                        vap(w4[:], 119 * 40, [[40, 40], [1, 40]]),
                        vap(w2[:], 119 * 80, [[80, 40], [1, 40]]))
                    nc.vector.tensor_tensor(
                        w5[:].rearrange("p (s c) -> p s c", c=20),
                        vap(w4[:], 0, [[40, S1], [1, 20]]),
                        vap(w4[:], 20, [[40, S1], [1, 20]]), Op.add)
                    nc.vector.tensor_tensor(
                        w6[:].rearrange("p (s c) -> p s c", c=10),
                        vap(w5[:], 0, [[20, S1], [1, 10]]),
                        vap(w5[:], 10, [[20, S1], [1, 10]]), Op.add)
                    nc.vector.tensor_reduce(dot16[:],
                                            vap(w6[:], 0, [[10, S1], [1, 10]]),
                                            mybir.AxisListType.X, Op.add)
                # --- theta = argmax dot*|dot|/nsq ---
                nc.vector.tensor_scalar(adot[:].bitcast(mybir.dt.uint16),
                                        dot16[:].bitcast(mybir.dt.uint16),
                                        0x7FFF, None, Op.bitwise_and)
                nc.vector.tensor_tensor(gsel[:], dot16[:], adot[:], Op.mult)
                nc.vector.tensor_tensor(gsel[:], gsel[:], rnsq[:], Op.mult)
                nc.vector.max(mx8[:], gsel[:])
                nc.vector.max_index(mi8[:], mx8[:], gsel[:])
                # --- y_align: scatter xpad16[79+j] -> yal[79+j-theta] ---
                nc.vector.scalar_tensor_tensor(ix1[:], io_t[:, 79:159],
                                               mi8[:, 0:1], io_t[:, 79:159],
                                               Op.subtract, Op.bypass)
                nc.vector.tensor_scalar(th2[:], mi8[:, 0:1], -1.0, 159.0,
                                        Op.mult, Op.add)
                nc.vector.scalar_tensor_tensor(ix2[:], io_t[:, 80:160],
                                               th2[:, 0:1], io_t[:, 80:160],
                                               Op.subtract, Op.bypass)
                nc.gpsimd.local_scatter(yal[:], xpad16[:, 79:159], ix1[:],
                                        channels=128, num_elems=256,
                                        num_idxs=80)
                # --- softmax attention -> y_att in yap16[:, 80:160] ---
                nc.vector.tensor_tensor(zt[:], yal[:, 0:80], yres[:], Op.mult)
                nc.scalar.activation(et[:], zt[:], AF.Exp,
                                     scale=1.0 / TEMPER,
                                     accum_out=ssum[:])
                nc.vector.reciprocal_approx_fast(rsum[:], ssum[:])
                nc.vector.scalar_tensor_tensor(yap16[:, 80:160], et[:],
                                                rsum[:, 0:1], yal[:, 0:80],
                                                Op.mult, Op.mult)
                # --- z features (fp16, packed 4096) ---
                foff = 0
                yb = yap16[:, 80:240]
                for d0, d1, im in ZBLOCKS:
                    nblk = (d1 - d0) * im
                    ov = bass.AP(zf16[:].tensor, zf16[:].offset + foff,
                                 [list(zf16[:].ap[0]), [im, d1 - d0], [1, im]])
                    b0 = bass.AP(yb.tensor, yb.offset,
                                 [list(yb.ap[0]), [0, d1 - d0], [1, im]])
                    b1 = bass.AP(yb.tensor, yb.offset + d0,
                                 [list(yb.ap[0]), [1, d1 - d0], [1, im]])
                    nc.vector.tensor_tensor(ov, b0, b1, Op.mult)
                    foff += nblk
                # --- E: transpose chunk-pairs -> one 256-wide copy (V/ACT
                #     alternating) -> token-major accumulate in PSUM ---
                nc.scalar.copy(zf16[:, NFEAT:NFEAT + 80], yap16[:, 80:160])
                zTs = pp.tile([128, 256], F32, tag="zTp")
                nc.tensor.matmul(zTs[:, 0:128], zf16[:, NFEAT:NFE], id_t,
                                 start=True, stop=True)
                zsbs = zpool.tile([128, 256], F16, tag="zT")
                nc.scalar.copy(zsbs[:, 0:128], zTs[:, 0:128])
                Eps = ppe.tile([128, 81], F32, tag="Eps")
                NP2 = NCH // 2  # 16 full pairs; chunk 32 went first
                nc.tensor.matmul(Eps[:], zsbs[:, 0:128], Achunk(2 * NP2),
                                 start=True, stop=False)
                zsb = [None] * (NP2 + 1)
                for k2 in range(NP2 + 1):
                    if k2 < NP2:
                        zTp = pp.tile([128, 256], F32, tag="zTp")
                        nc.tensor.matmul(zTp[:, 0:128],
                                         zf16[:, (2 * k2) * 128:(2 * k2 + 1) * 128],
                                         id_t, start=True, stop=True)
                        nc.tensor.matmul(zTp[:, 128:256],
                                         zf16[:, (2 * k2 + 1) * 128:(2 * k2 + 2) * 128],
                                         id_t, start=True, stop=True)
                        zsb_k = zpool.tile([128, 256], F16, tag="zT")
                        zsb[k2] = zsb_k
                        if k2 % 4 == 3:
                            nc.vector.tensor_copy(zsb[k2][:], zTp[:])
                        else:
                            nc.scalar.copy(zsb[k2][:], zTp[:])
                    j2 = k2 - 1
                    if 0 <= j2 < NP2:
                        nc.tensor.matmul(Eps[:], zsb[j2][:, 0:128],
                                         Achunk(2 * j2),
                                         start=False, stop=False)
                        nc.tensor.matmul(Eps[:], zsb[j2][:, 128:256],
                                         Achunk(2 * j2 + 1),
                                         start=False,
                                         stop=(j2 == NP2 - 1))
                # --- s* argmax directly on PSUM, d* = 80 - s* ---
                nc.vector.max(mx8[:], Eps[:])
                nc.vector.max_index(mi8[:], mx8[:], Eps[:])
                nc.vector.tensor_scalar(df[:], mi8[:, 0:1], -1.0, 80.0,
                                        Op.mult, Op.add)
                # --- yhat: scatter yap16[80+j] -> yhat[80+j-s*] ---
                nc.vector.scalar_tensor_tensor(ix3[:], io_t[:, 80:160],
                                               mi8[:, 0:1], io_t[:, 80:160],
                                               Op.subtract, Op.bypass)
                nc.gpsimd.local_scatter(yhat[:], yap16[:, 80:160], ix3[:],
                                        channels=128, num_elems=256,
                                        num_idxs=80)
                # x_ele scatter-back + next iteration's norms fill the
                # Vector-idle window during the enc/dec matmul chain
                nc.gpsimd.local_scatter(xele[:], yap16[:, 80:160], ix2[:],
                                        channels=128, num_elems=256,
                                        num_idxs=80)
                nc.vector.tensor_tensor(xpad[:, 79:159], xpad[:, 79:159],
                                        xele[:, 0:80], Op.subtract)
                if it + 1 < THINK_ITER:
                    norms()
                # --- x_extT = M @ yhat^T (+ bf), M = W_src @ W_enc ---
                yhTp = pph.tile([128, 128], F32, tag="Hp")
                nc.tensor.matmul(yhTp[:], yhat[:, 0:128], id_t,
                                 start=True, stop=True)
                nc.scalar.copy(yhT0[:], yhTp[:])
                yhTp2 = pph.tile([128, 128], F32, tag="Hp")
                nc.tensor.matmul(yhTp2[0:32, :], yhat[:, 128:160], id_t,
                                 start=True, stop=True)
                nc.scalar.copy(yhT1[:], yhTp2[0:32, :])
                for oc in range(2):
                    ow = 128 if oc == 0 else 32
                    Xp = pph.tile([128, 128], F32, tag="Hp")
                    nc.tensor.matmul(Xp[0:ow, :],
                                     c16[:, OF_M0 + oc * 128:OF_M0 + oc * 128 + ow],
                                     yhT0[:], start=True, stop=False)
                    nc.tensor.matmul(Xp[0:ow, :],
                                     c16[0:32, OF_M1 + oc * 128:OF_M1 + oc * 128 + ow],
                                     yhT1[:], start=False, stop=True)
                    dst = xeT0 if oc == 0 else xeT1
                    nc.scalar.activation(dst[:], Xp[0:ow, :], AF.Identity,
                                         bias=bs_t[0:ow, oc:oc + 1])
                Xtp = pph.tile([128, 128], F32, tag="Hp")
                nc.tensor.matmul(Xtp[:], xeT0[:], id_t, start=True, stop=True)
                nc.scalar.copy(xext16[:, 0:128], Xtp[:])
                Xtp2 = pph.tile([128, 128], F32, tag="Hp")
                nc.tensor.matmul(Xtp2[:, 0:32], xeT1[:], c16[0:32, OF_ID:OF_ID + 32],
                                 start=True, stop=True)
                nc.scalar.copy(xext16[:, 128:160], Xtp2[:, 0:32])
                # --- y_ele: scatter xext16[j] -> yele[j-d*] ---
                nc.vector.scalar_tensor_tensor(ix4[:], io_t[:, 0:160],
                                               df[:, 0:1], io_t[:, 0:160],
                                               Op.subtract, Op.bypass)
                nc.gpsimd.local_scatter(yele[:], xext16[:], ix4[:],
                                        channels=128, num_elems=160,
                                        num_idxs=160)
                # --- loss partial + state updates ---
                nc.vector.tensor_tensor(dtmp[:], yele[:, 0:80], yres[:],
                                        Op.subtract)
                nc.vector.tensor_tensor(dtmp[:], dtmp[:], keep[:], Op.mult)
                nc.scalar.activation(dsq[:], dtmp[:], AF.Square,
                                     accum_out=lossp[:, it:it + 1])
                nc.vector.tensor_tensor(yres[:], yres[:], yele[:, 0:80],
                                        Op.subtract)

            nc.sync.dma_start(d_out[:], lossp[:])
    return nc


def kernel(x, y, W_enc, b_enc, W_src, b_src):
    import sys
    if '/opt/trn_rl_repo' not in sys.path:
        sys.path.insert(0, '/opt/trn_rl_repo')
    x = np.asarray(x, np.float32)
    y = np.asarray(y, np.float32)
    consts = _build_consts(W_enc, b_enc, W_src, b_src)

    if "nc" not in _cache:
        _cache["nc"] = _build_nc()
        _cache["nc"].finalize()
    nc = _cache["nc"]

    in_maps = _make_in_maps(x, y, consts)
    from concourse.bass_utils import run_bass_kernel_spmd
    res = run_bass_kernel_spmd(nc, in_maps, list(range(NCORES)))
    parts = np.stack([r["losspart"] for r in res.results])
    keep_cnt = max(int((y != 0.0).sum()), 1)
    nums = parts[:, :, :THINK_ITER].sum(axis=(0, 1), dtype=np.float64)
    losses = (nums / keep_cnt).astype(np.float32)
    return np.float32(np.mean(losses))
